# revision 10
# baseline (speedup 1.0000x reference)
"""Trainium2 Bass kernel for windowed attention with dynamic position bias.

Shapes (hardcoded): qkv [3, 2, 65536, 192], H=W=256, window 8x32 (N=256),
6 heads, head_dim 32. 512 windows total, data-parallel over 8 cores
(64 windows each; each core owns a contiguous band of 64 H-rows of one batch).

v2 design (all-fp16 matmuls, host-side layout prep):
  Host pre-transposes Q^T/K^T per window into a single fp16 tile
  [96, 4*256] (blocks: Q heads0-2 | Q heads3-5 | K heads0-2 | K heads3-5),
  prebuilds V-extended fp16 [128, 408] (32 hd + ones col + pad, per kk),
  and the position bias as multiplicative M = exp(bias)*0.25 fp16
  [128, 2*1536] matching the score layout.

  Device per window (per head-triplet "half"):
    scores S^T[k,q] = K^T.T @ Q^T  (fp16 matmuls into PSUM [128,1536])
    P = exp(scale * S) on ACT -> SBUF fp16 (one instr per half)
    P2 = P * M on DVE (fp16, all-SBUF)
    AV (deferred one window for pipelining): out[q,:] += P2^T @ [V | 1]
    denominators from the ones column; DVE reciprocal + broadcast mult,
    fp16 out DMA; host casts to fp32 and un-windows.
"""
import sys
import numpy as np

sys.path.insert(0, "/opt/trn_rl_repo")

H_SP, W_SP = 8, 32
NUM_HEADS = 6
DIM = 192
HEAD_DIM = 32
N = H_SP * W_SP          # 256 tokens per window
LN_EPS = 1e-5
SCALE = HEAD_DIM ** -0.5
B, H, W = 2, 256, 256
L = H * W
N_CORES = 8
WINDOWS_PER_CORE = 64    # 8 hb bands x 8 wi
L_PER_CORE = L // 4      # 16384 tokens (64 H-rows)

_BUILT = None


def _np_layer_norm(x, g, b):
    m = x.mean(axis=-1, keepdims=True)
    v = ((x - m) ** 2).mean(axis=-1, keepdims=True)
    return (x - m) / np.sqrt(v + LN_EPS) * g + b


def _host_biasM(rpi, rpe_biases, p):
    """DynamicPosBias MLP + gather -> multiplicative bias exp(bias)*0.25,
    fp16 [128, 3072] laid out [half*1536 + (h_local*2+kk)*256 + q] with
    partition = k-local within the kk block."""
    x = rpe_biases.astype(np.float32)
    pos = x @ p["pos_proj_w"].T + p["pos_proj_b"]
    pos = np.maximum(_np_layer_norm(pos, p["ln1_g"], p["ln1_b"]), 0.0) @ p["fc1_w"].T + p["fc1_b"]
    pos = np.maximum(_np_layer_norm(pos, p["ln2_g"], p["ln2_b"]), 0.0) @ p["fc2_w"].T + p["fc2_b"]
    pos = np.maximum(_np_layer_norm(pos, p["ln3_g"], p["ln3_b"]), 0.0) @ p["fc3_w"].T + p["fc3_b"]
    # pos: [945, 6]; bias[h, q, k] = pos[rpi[q, k], h]
    rel = pos[np.asarray(rpi).reshape(-1)].reshape(N, N, NUM_HEADS)  # [q, k, h]
    M = np.exp(rel, dtype=np.float32) * np.float32(0.25)
    biasM = np.empty((128, 2 * 1536), dtype=np.float16)
    for half in range(2):
        for hl in range(3):
            h = 3 * half + hl
            mt = M[:, :, h].T                      # [k, q]
            for kk in range(2):
                col = half * 1536 + (hl * 2 + kk) * 256
                biasM[:, col:col + 256] = mt[kk * 128:(kk + 1) * 128, :]
    return biasM


def _build():
    import concourse.bass as bass  # noqa: F401
    import concourse.mybir as mybir
    import concourse.tile as tile
    from concourse import bacc

    dt = mybir.dt
    nc = bacc.Bacc("TRN2", target_bir_lowering=False, debug=False)
    qk_in = nc.declare_dram_parameter("qk_c", [WINDOWS_PER_CORE, 96, 1024], dt.float16, isOutput=False)
    vext_in = nc.declare_dram_parameter("vext_c", [WINDOWS_PER_CORE, 128, 408], dt.float16, isOutput=False)
    biasM_in = nc.declare_dram_parameter("biasM", [128, 3072], dt.float16, isOutput=False)
    out_c = nc.declare_dram_parameter("out_c", [WINDOWS_PER_CORE, 128, 2, 192], dt.float16, isOutput=True)

    with tile.TileContext(nc) as tc:
        with (
            tc.tile_pool(name="const", bufs=1) as cp,
            tc.tile_pool(name="io_qk", bufs=3) as io_qk,
            tc.tile_pool(name="io_v", bufs=3) as io_v,
            tc.tile_pool(name="ptp", bufs=4) as ptp,
            tc.tile_pool(name="pt2p", bufs=4) as pt2p,
            tc.tile_pool(name="op", bufs=6) as op,
            tc.tile_pool(name="ps_s", bufs=2, space="PSUM") as ps_s,
            tc.tile_pool(name="ps_av", bufs=2, space="PSUM") as ps_av,
        ):
            biasM = cp.tile([128, 3072], dt.float16, tag="biasM")
            nc.gpsimd.dma_start(out=biasM[:], in_=biasM_in[:])

            def emit_av_half(psav, pt2h, vextp, half):
                for hl in range(3):
                    h = 3 * half + hl
                    for qc in range(2):
                        for kk in range(2):
                            j = hl * 2 + kk
                            nc.tensor.matmul(
                                psav[:, (qc * 6 + h) * 34:(qc * 6 + h) * 34 + 34],
                                pt2h[:, j * 256 + qc * 128: j * 256 + qc * 128 + 128],
                                vextp[:, kk * 204 + h * 34: kk * 204 + (h + 1) * 34],
                                start=(kk == 0), stop=(kk == 1), skip_group_check=True)

            def emit_norm(prev, psav):
                wp, pt2s, vextp = prev
                den = psav[:].rearrange("p (x c) -> p x c", x=12, c=34)[:, :, 32:33]
                rec = op.tile([128, 12], dt.float32, tag="rec")
                nc.vector.reciprocal(rec[:], den)
                osb = op.tile([128, 384], dt.float16, tag="osb")
                av = psav[:].rearrange("p (x c) -> p x c", x=12, c=34)[:, :, 0:32]
                nc.vector.tensor_tensor(
                    out=osb[:].rearrange("p (x c) -> p x c", x=12, c=32),
                    in0=av,
                    in1=rec[:].broadcast_to([128, 12, 32]),
                    op=mybir.AluOpType.mult)
                nc.gpsimd.dma_start(
                    out=out_c[wp].rearrange("p qc c -> p (qc c)"), in_=osb[:])

            def emit_flush(prev):
                wp, pt2s, vextp = prev
                psav = ps_av.tile([128, 408], dt.float32, tag="av")
                emit_av_half(psav, pt2s[0], vextp, 0)
                emit_av_half(psav, pt2s[1], vextp, 1)
                emit_norm(prev, psav)

            prev = None
            for w in range(WINDOWS_PER_CORE):
                # qk col blocks: [Q_A | K_A | Q_B | K_B]
                qk = io_qk.tile([96, 1024], dt.float16, tag="qk")
                nc.sync.dma_start(out=qk[:], in_=qk_in[w])
                vext = io_v.tile([128, 408], dt.float16, tag="vext")
                nc.gpsimd.dma_start(out=vext[:], in_=vext_in[w])

                pt2s = []
                for half in range(2):
                    pss = ps_s.tile([128, 1536], dt.float32, tag="scores")
                    for hl in range(3):
                        for kk in range(2):
                            j = hl * 2 + kk
                            nc.tensor.matmul(
                                pss[:, j * 256:(j + 1) * 256],
                                qk[32 * hl:32 * hl + 32,
                                   (half * 2 + 1) * 256 + kk * 128:(half * 2 + 1) * 256 + kk * 128 + 128],
                                qk[32 * hl:32 * hl + 32, half * 512:half * 512 + 256],
                                start=True, stop=True, skip_group_check=True)
                    pt = ptp.tile([128, 1536], dt.float16, tag="pt")
                    nc.scalar.activation(pt[:], pss[:], mybir.ActivationFunctionType.Exp,
                                         scale=float(SCALE))
                    pt2 = pt2p.tile([128, 1536], dt.float16, tag="pt2")
                    nc.vector.tensor_tensor(out=pt2[:], in0=pt[:],
                                            in1=biasM[:, half * 1536:(half + 1) * 1536],
                                            op=mybir.AluOpType.mult)
                    pt2s.append(pt2)

                if prev is not None:
                    emit_flush(prev)
                prev = (w, pt2s, vext)
            emit_flush(prev)
    nc.compile()
    return nc


def _get_nc():
    global _BUILT
    if _BUILT is None:
        _BUILT = _build()
    return _BUILT


def kernel(qkv, H, W, rpi, rpe_biases, pos_proj_w, pos_proj_b, ln1_g, ln1_b,
           fc1_w, fc1_b, ln2_g, ln2_b, fc2_w, fc2_b, ln3_g, ln3_b,
           fc3_w, fc3_b, _trace=False):
    from concourse.bass_utils import run_bass_kernel_spmd

    qkv = np.asarray(qkv, dtype=np.float32)
    params = dict(pos_proj_w=pos_proj_w, pos_proj_b=pos_proj_b, ln1_g=ln1_g,
                  ln1_b=ln1_b, fc1_w=fc1_w, fc1_b=fc1_b, ln2_g=ln2_g,
                  ln2_b=ln2_b, fc2_w=fc2_w, fc2_b=fc2_b, ln3_g=ln3_g,
                  ln3_b=ln3_b, fc3_w=fc3_w, fc3_b=fc3_b)
    params = {k: np.asarray(v, dtype=np.float32) for k, v in params.items()}
    biasM = _host_biasM(rpi, rpe_biases, params)

    nc = _get_nc()
    in_maps = []
    for c in range(N_CORES):
        b = c // 4
        row0 = (c % 4) * L_PER_CORE
        blk = qkv[:, b, row0:row0 + L_PER_CORE, :]  # [3, 16384, 192]
        # [hb, h, wi, j, C] -> windows [w, tok, C]
        win = blk.reshape(3, 8, 8, 8, 32, DIM).transpose(0, 1, 3, 2, 4, 5).reshape(
            3, WINDOWS_PER_CORE, N, DIM)
        qT = win[0].transpose(0, 2, 1).reshape(WINDOWS_PER_CORE, 6, 32, 256)
        kT = win[1].transpose(0, 2, 1).reshape(WINDOWS_PER_CORE, 6, 32, 256)
        qk_c = np.stack(
            [qT[:, 0:3].reshape(-1, 96, 256), kT[:, 0:3].reshape(-1, 96, 256),
             qT[:, 3:6].reshape(-1, 96, 256), kT[:, 3:6].reshape(-1, 96, 256)],
            axis=2).astype(np.float16).reshape(WINDOWS_PER_CORE, 96, 1024)
        vwin = win[2].reshape(WINDOWS_PER_CORE, 2, 128, 6, 32)  # [w, kk, p, h, hd]
        tmp = np.zeros((WINDOWS_PER_CORE, 2, 128, 6, 34), dtype=np.float16)
        tmp[..., :32] = vwin
        tmp[..., 32] = 1.0
        vext_c = np.ascontiguousarray(tmp.transpose(0, 2, 1, 3, 4)).reshape(
            WINDOWS_PER_CORE, 128, 408)
        in_maps.append({
            "qk_c": np.ascontiguousarray(qk_c),
            "vext_c": vext_c,
            "biasM": biasM,
        })
    res = run_bass_kernel_spmd(nc, in_maps, list(range(N_CORES)), trace=_trace)
    out = np.empty((B, H, W, DIM), dtype=np.float32)
    for c in range(N_CORES):
        b = c // 4
        h0 = (c % 4) * 64
        x = res.results[c]["out_c"].astype(np.float32)  # [64, 128, 2, 192]
        x = x.transpose(0, 2, 1, 3).reshape(8, 8, 8, 32, DIM)  # tok = qc*128+p
        # [hb, wi, h, j, C] -> [hb, h, wi, j, C] -> [64, 256, C]
        out[b, h0:h0 + 64, :, :] = x.transpose(0, 2, 1, 3, 4).reshape(64, W, DIM)
    if _trace:
        return out, res
    return out
